# revision 12
# baseline (speedup 1.0000x reference)
"""CASTLE forward kernel for 8 Trainium2 NeuronCores.

Shards the num_inputs (branch) axis: core c owns branches [8c, 8c+8).
x and the shared Linear are replicated; each core owns its slice of
mask_w / mask_b / W_out / b_out and the [B, I_local, H] activations.

Key algebra: x_i = h0 @ W_share + b_share is never an output — only its
O=2-dim projection through W_out is.  So the big [B,H]x[H,H] stage is
replaced by per-branch effective weights computed once:
    Weff_i = W_share @ W_out[i]            [512, 2]
    beff_i = b_share @ W_out[i] + b_out[i] [2]
    out_layer_i = h0_i @ Weff_i + beff_i
and softmax over 2 classes == sigmoid of the logit difference.

Per-core pipeline (branch i, local il):
  stage A: h0T = gelu([masked_i; mask_b_i]^T @ [x; 1]^T)  (bias as 65th row)
  stage C: dT_i = (Weff_i[:,1]-Weff_i[:,0])^T @ h0T       [1, B]
  end:     Out row il = sigmoid(dT_i + dbias_i), batched over branches

All matmuls in float32r (single-pass PE mode); all sigmoids are batched
after all gelus so ACT LUT-table reloads collapse to 3 (enforced with
explicit scheduler dep edges).
"""

import os
import sys

for _p in ("/opt/trn_rl_repo", "/root/.axon_site/_ro/trn_rl_repo"):
    if os.path.isdir(_p) and _p not in sys.path:
        sys.path.insert(0, _p)

import numpy as np

I, H, O, B = 64, 512, 2, 2048
N_CORES, IL = 8, 8          # cores, local branches per core
EPS = 1e-12
BT = 512                    # batch tile (fp32 moving-operand max)
NBT = B // BT
NHC = H // 128              # 128-row chunks of the hidden dim

_CACHE = {}


# ---------------------------------------------------------------- build

def _build(repeats=1, const_ins=None):
    """const_ins: timing-only mode — bake the (core-0) input arrays into the
    NEFF as Const tensors and keep only a tiny chain token + OutT as real
    I/O, so per-exec relay traffic (and its timing jitter) collapses."""
    import concourse.tile as tile
    from concourse import bacc, mybir
    from concourse.bass import _add_dep_helper

    f32, f32r = mybir.dt.float32, mybir.dt.float32r
    AF = mybir.ActivationFunctionType

    nc = bacc.Bacc("TRN2", target_bir_lowering=False, debug=False,
                   num_devices=N_CORES)

    timing = const_ins is not None

    def d_in(name, shape):
        if timing:
            return nc.inline_tensor(
                np.ascontiguousarray(const_ins[name], np.float32), name=name)
        return nc.dram_tensor(name, shape, f32, kind="ExternalInput")

    def d_out(name, shape, keep=False):
        if timing and not keep:
            return nc.dram_tensor(name, shape, f32)  # internal DRAM scratch
        return nc.dram_tensor(name, shape, f32, kind="ExternalOutput")

    # -------- per-core DRAM I/O (SPMD: same shapes, per-core data)
    # x65 = [x^T; ones] so mask_b rides the contraction as a 65th row
    x65_d = d_in("x65", [I + 1, B])
    mwT_d = d_in("mwT", [I, IL, H])
    mb1_d = d_in("mb1", [1, IL, H])
    dsc_d = d_in("dscale", [I, IL])
    # W_share^T packed for the Weff matmuls: WsTT[p, kc, n] = Ws[n, kc*128+p]
    WsTT_d = d_in("WsTT", [128, NHC, H])
    bshT_d = d_in("bshT", [128, NHC])
    WoT_d = d_in("WoT", [128, IL, NHC, O])
    bod8_d = d_in("bod8", [IL, 1])
    bo01_d = d_in("bo01", [O, IL])

    masked_d = d_out("masked_o", [I, IL, H])
    WT_d = d_out("WT", [I, IL])
    OutT_d = d_out("OutT", [IL, B], keep=True)
    OL0_d = d_out("OL0", [O, B])
    OSM0_d = d_out("OSM0", [O, B])
    if timing:
        tok_d = nc.dram_tensor("tok", [1, IL], f32, kind="ExternalInput")
        tok_o = nc.dram_tensor("tok_o", [1, IL], f32, kind="ExternalOutput")

    with tile.TileContext(nc) as tc:
        with (
            tc.tile_pool(name="consts", bufs=1) as consts,
            tc.tile_pool(name="h0", bufs=3) as h0_pool,
            tc.tile_pool(name="dall", bufs=6) as dall_pool,
            tc.tile_pool(name="small", bufs=4) as small,
            tc.tile_pool(name="psA", bufs=2, space="PSUM") as psA,
            tc.tile_pool(name="psWF", bufs=1, space="PSUM") as psWF,
            tc.tile_pool(name="psC", bufs=2, space="PSUM") as psC,
            tc.tile_pool(name="psL", bufs=1, space="PSUM") as psL,
        ):
            # -------- load constants
            x65_s = consts.tile([I + 1, B], f32)
            nc.sync.dma_start(x65_s[:], x65_d[:])
            x65r = consts.tile([I + 1, B], f32r)
            nc.vector.tensor_copy(x65r[:], x65_s[:])

            # mw65[0:64] = mask_w slice, row 64 = mask_b (the bias row)
            mw65_s = consts.tile([I + 1, IL, H], f32)
            nc.sync.dma_start(mw65_s[0:I, :, :], mwT_d[:])
            nc.sync.dma_start(mw65_s[I:I + 1, :, :], mb1_d[:])
            dsc_s = consts.tile([I, IL], f32)
            nc.sync.dma_start(dsc_s[:], dsc_d[:])

            WsTT_s = consts.tile([128, NHC, H], f32)
            nc.sync.dma_start(WsTT_s[:], WsTT_d[:])
            WsTTr = consts.tile([128, NHC, H], f32r)
            nc.vector.tensor_copy(WsTTr[:], WsTT_s[:])
            bsh_s = consts.tile([128, NHC], f32)
            nc.sync.dma_start(bsh_s[:], bshT_d[:])
            # fp32r matmuls need N>=2: duplicate b_share into two rhs cols
            bshr2 = consts.tile([128, NHC, 2], f32r)
            bshc = bsh_s.rearrange("p (n o) -> p n o", o=1)
            nc.vector.tensor_copy(bshr2[:, :, 0:1], bshc)
            nc.vector.tensor_copy(bshr2[:, :, 1:2], bshc)

            Wo_s = consts.tile([128, IL, NHC, O], f32)
            nc.sync.dma_start(Wo_s[:], WoT_d[:])
            Wor = consts.tile([128, IL, NHC, O], f32r)
            nc.vector.tensor_copy(Wor[:], Wo_s[:])
            # logit-diff weights over the m dim (softmax-of-2 trick)
            Wod = consts.tile([128, IL, NHC], f32r)
            nc.vector.tensor_sub(Wod[:], Wo_s[:, :, :, 1], Wo_s[:, :, :, 0])

            bod8_s = consts.tile([IL, 1], f32)
            nc.sync.dma_start(bod8_s[:], bod8_d[:])
            bo01_s = consts.tile([O, IL], f32)
            nc.sync.dma_start(bo01_s[:], bo01_d[:])

            # -------- masked weights (scale diagonal rows), W column norms
            mw65r = consts.tile([I + 1, IL, H], f32r)
            wsq = consts.tile([I, IL], f32)
            sq_scr = consts.tile([I, H], f32)
            for il in range(IL):
                nc.vector.tensor_scalar_mul(
                    mw65_s[0:I, il, :], mw65_s[0:I, il, :],
                    dsc_s[:, il:il + 1])
                nc.sync.dma_start(masked_d[:, il, :], mw65_s[0:I, il, :])
                nc.scalar.activation(sq_scr[:], mw65_s[0:I, il, :], AF.Square,
                                     accum_out=wsq[:, il:il + 1])
            nc.vector.tensor_copy(mw65r[:], mw65_s[:])

            # -------- Weff = W_share @ W_out (all 8 branches: N=16 rhs)
            weff_r = consts.tile([128, NHC, IL * O], f32r)
            for hc in range(NHC):
                pw = psWF.tile([128, IL * O], f32, tag="wf")
                for kc in range(NHC):
                    nc.tensor.matmul(
                        pw[:], WsTTr[:, kc, hc * 128:(hc + 1) * 128],
                        Wor[:, :, kc, :], start=(kc == 0),
                        stop=(kc == NHC - 1))
                nc.vector.tensor_copy(weff_r[:, hc, :], pw[:])
            # per-branch logit-diff columns of Weff
            weff4 = weff_r.rearrange("p n (i o) -> p n i o", o=O)
            weffd = consts.tile([128, NHC, IL], f32r)
            nc.vector.tensor_sub(weffd[:], weff4[:, :, :, 1], weff4[:, :, :, 0])

            # -------- bias folds: dbias = b_share @ Wod + (bo1 - bo0);
            #          beff0 = b_share @ W_out[0] + b_out[0]
            pb8 = psWF.tile([IL, 2], f32, tag="wf")
            for kc in range(NHC):
                nc.tensor.matmul(pb8[:], Wod[:, :, kc], bshr2[:, kc, :],
                                 start=(kc == 0), stop=(kc == NHC - 1))
            dbias = consts.tile([IL, 1], f32)
            nc.vector.tensor_add(dbias[:], pb8[:, 0:1], bod8_s[:])

            pb0 = psWF.tile([O, 2], f32, tag="wf")
            for kc in range(NHC):
                nc.tensor.matmul(pb0[:], Wor[:, 0, kc, :], bshr2[:, kc, :],
                                 start=(kc == 0), stop=(kc == NHC - 1))
            beff0 = consts.tile([O, 1], f32)
            nc.vector.tensor_add(beff0[:], pb0[:, 0:1], bo01_s[:, 0:1])

            # -------- main batched pipeline (gelu table on ACT throughout)
            dstack = consts.tile([IL, B], f32)
            last_gelu = None
            for rep in range(repeats):
                for bt in range(NBT):
                    bsl = slice(bt * BT, (bt + 1) * BT)
                    for il in range(IL):
                        # stage A: h0T chunks, two PSUM banks per ACT op
                        h0_sb = h0_pool.tile([128, NHC, BT], f32r, tag="h0")
                        for hp in range(NHC // 2):
                            pa = psA.tile([128, 2, BT], f32, tag="psA")
                            for h2 in range(2):
                                hc = 2 * hp + h2
                                nc.tensor.matmul(
                                    pa[:, h2, :],
                                    mw65r[:, il, hc * 128:(hc + 1) * 128],
                                    x65r[:, bsl], start=True, stop=True)
                            g_i = nc.scalar.activation(
                                h0_sb[:, 2 * hp:2 * hp + 2, :], pa[:],
                                AF.Gelu)
                            last_gelu = g_i
                        # stage C: logit diff directly from h0
                        pd = psC.tile([1, BT], f32, tag="psC")
                        for kc in range(NHC):
                            nc.tensor.matmul(
                                pd[:], weffd[:, kc, il:il + 1],
                                h0_sb[:, kc, :],
                                start=(kc == 0), stop=(kc == NHC - 1))
                        dal = dall_pool.tile([1, BT], f32, tag="dall")
                        nc.vector.tensor_copy(dal[:], pd[:])
                        # scatter to the branch's partition for batched sigmoid
                        nc.sync.dma_start(dstack[il:il + 1, bsl], dal[:])
                        if il == 0:
                            # branch-0 raw logits (out_layer[:, 0, :])
                            pl = psL.tile([O, BT], f32, tag="psL")
                            for kc in range(NHC):
                                nc.tensor.matmul(
                                    pl[:], weff4[:, kc, 0, :],
                                    h0_sb[:, kc, :],
                                    start=(kc == 0), stop=(kc == NHC - 1))
                            ol = small.tile([O, BT], f32, tag="ol")
                            nc.vector.tensor_scalar_add(
                                ol[:], pl[:], beff0[:, 0:1])
                            nc.sync.dma_start(OL0_d[:, bsl], ol[:])

            # -------- end phase: batched sigmoids (one table swap), sqrt
            def after_gelus(inst):
                _add_dep_helper(inst.ins, last_gelu.ins, sync=True,
                                reason="batch sigmoids after gelus")

            os_t = consts.tile([IL, B], f32)
            s_i = nc.scalar.activation(os_t[:], dstack[:], AF.Sigmoid,
                                       bias=dbias[:])
            after_gelus(s_i)
            nc.sync.dma_start(OutT_d[:], os_t[:])
            nc.sync.dma_start(OSM0_d[1:2, :], os_t[0:1, :])
            om0 = consts.tile([1, B], f32)
            nc.vector.tensor_scalar(
                out=om0[:], in0=os_t[0:1, :], scalar1=-1.0, scalar2=1.0,
                op0=mybir.AluOpType.mult, op1=mybir.AluOpType.add)
            nc.sync.dma_start(OSM0_d[0:1, :], om0[:])

            wt_s = consts.tile([I, IL], f32)
            sq_i = nc.scalar.activation(wt_s[:], wsq[:], AF.Sqrt)
            after_gelus(sq_i)
            nc.sync.dma_start(WT_d[:], wt_s[:])

            if timing:
                tok_s = consts.tile([1, IL], f32)
                nc.sync.dma_start(tok_s[:], tok_d[:])
                nc.sync.dma_start(tok_o[:], tok_s[:])

    nc.compile()
    return nc


# ---------------------------------------------------------------- run

def _make_runner(nc):
    """jit-once runner: takes list of per-core input dicts, returns list of
    per-core output dicts. Modeled on bass2jax.run_bass_via_pjrt."""
    import jax
    from jax.sharding import Mesh, PartitionSpec
    from jax.experimental.shard_map import shard_map
    import concourse.mybir as mybir
    from concourse.bass2jax import (_bass_exec_p, install_neuronx_cc_hook,
                                    partition_id_tensor)

    install_neuronx_cc_hook()

    part_name = nc.partition_id_tensor.name if nc.partition_id_tensor else None
    in_names, out_names, out_avals = [], [], []
    for alloc in nc.m.functions[0].allocations:
        if not isinstance(alloc, mybir.MemoryLocationSet):
            continue
        name = alloc.memorylocations[0].name
        if alloc.kind == "ExternalInput":
            if name != part_name:
                in_names.append(name)
        elif alloc.kind == "ExternalOutput":
            out_names.append(name)
            out_avals.append(jax.core.ShapedArray(
                tuple(alloc.tensor_shape), mybir.dt.np(alloc.dtype)))
    n_params = len(in_names)
    all_names = in_names + out_names + ([part_name] if part_name else [])

    def _body(*args):
        operands = list(args)
        if part_name is not None:
            operands.append(partition_id_tensor())
        outs = _bass_exec_p.bind(
            *operands, out_avals=tuple(out_avals), in_names=tuple(all_names),
            out_names=tuple(out_names), lowering_input_output_aliases=(),
            sim_require_finite=True, sim_require_nnan=True, nc=nc)
        return tuple(outs)

    devices = jax.devices()[:N_CORES]
    mesh = Mesh(np.asarray(devices), ("core",))
    n_outs = len(out_names)
    sharded = jax.jit(shard_map(
        _body, mesh=mesh,
        in_specs=(PartitionSpec("core"),) * (n_params + n_outs),
        out_specs=(PartitionSpec("core"),) * n_outs, check_rep=False))

    zero_shapes = [tuple(a.shape) for a in out_avals]

    def run(in_maps):
        concat_in = [np.concatenate([np.asarray(in_maps[c][n])
                                     for c in range(N_CORES)], axis=0)
                     for n in in_names]
        concat_zeros = [np.zeros((N_CORES * s[0],) + s[1:], np.float32)
                        for s in zero_shapes]
        outs = sharded(*concat_in, *concat_zeros)
        return [
            {name: np.asarray(outs[i]).reshape((N_CORES,) + zero_shapes[i])[c]
             for i, name in enumerate(out_names)}
            for c in range(N_CORES)
        ]

    run.in_names = in_names
    run.out_names = out_names
    run.sharded = sharded
    run.n_params = n_params
    run.zero_shapes = zero_shapes
    return run


def _get_runner(repeats=1):
    key = ("runner", repeats)
    if key not in _CACHE:
        _CACHE[key] = _make_runner(_build(repeats))
    return _CACHE[key]


def _get_timing_runner(repeats, in_map0):
    key = ("timing", repeats)
    if key not in _CACHE:
        _CACHE[key] = _make_runner(_build(repeats, const_ins=in_map0))
    return _CACHE[key]


# ---------------------------------------------------------------- host glue

def _prep_inputs(x, mask_w, mask_b, W_share, b_share, W_out, b_out):
    f = np.float32
    x = np.asarray(x, f)
    mask_w = np.asarray(mask_w, f)
    mask_b = np.asarray(mask_b, f)
    W_share = np.asarray(W_share, f)
    b_share = np.asarray(b_share, f)
    W_out = np.asarray(W_out, f)
    b_out = np.asarray(b_out, f)

    x65 = np.concatenate([x.T, np.ones((1, B), f)], axis=0)          # [65, B]
    WsTT = np.ascontiguousarray(
        W_share.T.reshape(NHC, 128, H).transpose(1, 0, 2))           # [128, 4, 512]
    bshT = np.ascontiguousarray(b_share.reshape(NHC, 128).T)         # [128, 4]

    in_maps = []
    for c in range(N_CORES):
        sl = slice(c * IL, (c + 1) * IL)
        mwT = np.ascontiguousarray(mask_w[sl].transpose(1, 0, 2))    # [64, 8, 512]
        mb1 = np.ascontiguousarray(mask_b[sl].reshape(1, IL, H))     # [1, 8, 512]
        dsc = np.ones((I, IL), f)
        for il in range(IL):
            dsc[c * IL + il, il] = EPS
        WoT = np.ascontiguousarray(
            W_out[sl].reshape(IL, NHC, 128, O).transpose(2, 0, 1, 3))  # [128,8,4,2]
        bod8 = (b_out[sl, 1] - b_out[sl, 0]).reshape(IL, 1).astype(f)
        in_maps.append({
            "x65": x65, "mwT": mwT, "mb1": mb1, "dscale": dsc,
            "WsTT": WsTT, "bshT": bshT, "WoT": WoT,
            "bod8": bod8,
            "bo01": np.ascontiguousarray(b_out[sl].T),               # [2, 8]
        })
    return in_maps


def _assemble(results):
    masked = np.empty((I, I, H), np.float32)
    for c in range(N_CORES):
        masked[c * IL:(c + 1) * IL] = results[c]["masked_o"].transpose(1, 0, 2)
    W = np.concatenate([results[c]["WT"] for c in range(N_CORES)], axis=1)
    Out = np.concatenate([results[c]["OutT"] for c in range(N_CORES)],
                         axis=0).T.copy()
    ol0 = np.ascontiguousarray(results[0]["OL0"].T)
    osm0 = np.ascontiguousarray(results[0]["OSM0"].T)
    return W, Out, ol0, masked, osm0


def kernel(x, mask_w, mask_b, W_share, b_share, W_out, b_out):
    in_maps = _prep_inputs(x, mask_w, mask_b, W_share, b_share, W_out, b_out)
    run = _get_runner(repeats=1)
    results = run(in_maps)
    return _assemble(results)


# revision 14
# speedup vs baseline: 2.7436x; 2.7436x over previous
"""CASTLE forward kernel for 8 Trainium2 NeuronCores.

Shards the num_inputs (branch) axis: core c owns branches [8c, 8c+8).
x and the shared Linear are replicated; each core owns its slice of
mask_w / mask_b / W_out / b_out and the [B, I_local, H] activations.

Key algebra: x_i = h0 @ W_share + b_share is never an output — only its
O=2-dim projection through W_out is.  So the big [B,H]x[H,H] stage is
replaced by per-branch effective weights computed once:
    Weff_i = W_share @ W_out[i]            [512, 2]
    beff_i = b_share @ W_out[i] + b_out[i] [2]
    out_layer_i = h0_i @ Weff_i + beff_i
and softmax over 2 classes == sigmoid of the logit difference.

Per-core pipeline (branch i, local il):
  stage A: h0T = gelu([masked_i; mask_b_i]^T @ [x; 1]^T)  (bias as 65th row)
  stage C: dT_i = (Weff_i[:,1]-Weff_i[:,0])^T @ h0T       [1, B]
  end:     Out row il = sigmoid(dT_i + dbias_i), batched over branches

All matmuls in float32r (single-pass PE mode); all sigmoids are batched
after all gelus so ACT LUT-table reloads collapse to 3 (enforced with
explicit scheduler dep edges).
"""

import os
import sys

for _p in ("/opt/trn_rl_repo", "/root/.axon_site/_ro/trn_rl_repo"):
    if os.path.isdir(_p) and _p not in sys.path:
        sys.path.insert(0, _p)

import numpy as np

I, H, O, B = 64, 512, 2, 2048
N_CORES, IL = 8, 8          # cores, local branches per core
EPS = 1e-12
BT = 512                    # batch tile (fp32 moving-operand max)
NBT = B // BT
NHC = H // 128              # 128-row chunks of the hidden dim

_CACHE = {}


# ---------------------------------------------------------------- build

def _build(repeats=1, const_ins=None):
    """const_ins: timing-only mode — bake the (core-0) input arrays into the
    NEFF as Const tensors and keep only a tiny chain token + OutT as real
    I/O, so per-exec relay traffic (and its timing jitter) collapses."""
    import concourse.tile as tile
    from concourse import bacc, mybir
    from concourse.bass import _add_dep_helper

    f32, f32r = mybir.dt.float32, mybir.dt.float32r
    AF = mybir.ActivationFunctionType

    nc = bacc.Bacc("TRN2", target_bir_lowering=False, debug=False,
                   num_devices=N_CORES)

    timing = const_ins is not None

    def d_in(name, shape):
        if timing:
            return nc.inline_tensor(
                np.ascontiguousarray(const_ins[name], np.float32), name=name)
        return nc.dram_tensor(name, shape, f32, kind="ExternalInput")

    def d_out(name, shape, keep=False):
        if timing and not keep:
            return nc.dram_tensor(name, shape, f32)  # internal DRAM scratch
        return nc.dram_tensor(name, shape, f32, kind="ExternalOutput")

    # -------- per-core DRAM I/O (SPMD: same shapes, per-core data)
    # x65 = [x^T; ones] so mask_b rides the contraction as a 65th row
    x65_d = d_in("x65", [I + 1, B])
    mwT_d = d_in("mwT", [I, IL, H])
    mb1_d = d_in("mb1", [1, IL, H])
    dsc_d = d_in("dscale", [I, IL])
    # W_share^T packed for the Weff matmuls: WsTT[p, kc, n] = Ws[n, kc*128+p]
    WsTT_d = d_in("WsTT", [128, NHC, H])
    bshT_d = d_in("bshT", [128, NHC])
    WoT_d = d_in("WoT", [128, IL, NHC, O])
    bod8_d = d_in("bod8", [IL, 1])
    bo01_d = d_in("bo01", [O, IL])

    masked_d = d_out("masked_o", [I, IL, H])
    WT_d = d_out("WT", [I, IL])
    OutT_d = d_out("OutT", [IL, B], keep=True)
    OL0_d = d_out("OL0", [O, B])
    OSM0_d = d_out("OSM0", [O, B])
    if timing:
        tok_d = nc.dram_tensor("tok", [1, IL], f32, kind="ExternalInput")
        tok_o = nc.dram_tensor("tok_o", [1, IL], f32, kind="ExternalOutput")

    with tile.TileContext(nc) as tc:
        with (
            tc.tile_pool(name="consts", bufs=1) as consts,
            tc.tile_pool(name="h0", bufs=3) as h0_pool,
            tc.tile_pool(name="dall", bufs=6) as dall_pool,
            tc.tile_pool(name="small", bufs=4) as small,
            tc.tile_pool(name="psC", bufs=1, space="PSUM") as psC,
            tc.tile_pool(name="psL", bufs=1, space="PSUM") as psL,
        ):
            # -------- load constants
            x65_s = consts.tile([I + 1, B], f32)
            nc.sync.dma_start(x65_s[:], x65_d[:])
            x65r = consts.tile([I + 1, B], f32r)
            nc.vector.tensor_copy(x65r[:], x65_s[:])

            # mw65[0:64] = mask_w slice, row 64 = mask_b (the bias row)
            mw65_s = consts.tile([I + 1, IL, H], f32)
            for il in range(IL):
                nc.sync.dma_start(mw65_s[0:I, il, :], mwT_d[:, il, :])
            nc.sync.dma_start(mw65_s[I:I + 1, :, :], mb1_d[:])
            dsc_s = consts.tile([I, IL], f32)
            nc.sync.dma_start(dsc_s[:], dsc_d[:])

            WsTT_s = consts.tile([128, NHC, H], f32)
            nc.sync.dma_start(WsTT_s[:], WsTT_d[:])
            WsTTr = consts.tile([128, NHC, H], f32r)
            nc.vector.tensor_copy(WsTTr[:], WsTT_s[:])
            bsh_s = consts.tile([128, NHC], f32)
            nc.sync.dma_start(bsh_s[:], bshT_d[:])
            # fp32r matmuls need N>=2: duplicate b_share into two rhs cols
            bshr2 = consts.tile([128, NHC, 2], f32r)
            bshc = bsh_s.rearrange("p (n o) -> p n o", o=1)
            nc.vector.tensor_copy(bshr2[:, :, 0:1], bshc)
            nc.vector.tensor_copy(bshr2[:, :, 1:2], bshc)

            Wo_s = consts.tile([128, IL, NHC, O], f32)
            nc.sync.dma_start(Wo_s[:], WoT_d[:])
            Wor = consts.tile([128, IL, NHC, O], f32r)
            nc.vector.tensor_copy(Wor[:], Wo_s[:])
            # logit-diff weights over the m dim (softmax-of-2 trick)
            Wod = consts.tile([128, IL, NHC], f32r)
            nc.vector.tensor_sub(Wod[:], Wo_s[:, :, :, 1], Wo_s[:, :, :, 0])

            bod8_s = consts.tile([IL, 1], f32)
            nc.sync.dma_start(bod8_s[:], bod8_d[:])
            bo01_s = consts.tile([O, IL], f32)
            nc.sync.dma_start(bo01_s[:], bo01_d[:])

            # -------- masked weights (scale diagonal rows), W column norms
            mw65r = consts.tile([I + 1, IL, H], f32r)
            wsq = consts.tile([I, IL], f32)
            sq_scr = consts.tile([I, H], f32)
            for il in range(IL):
                nc.vector.tensor_scalar_mul(
                    mw65_s[0:I, il, :], mw65_s[0:I, il, :],
                    dsc_s[:, il:il + 1])
                nc.sync.dma_start(masked_d[:, il, :], mw65_s[0:I, il, :])
                nc.scalar.activation(sq_scr[:], mw65_s[0:I, il, :], AF.Square,
                                     accum_out=wsq[:, il:il + 1])
                nc.vector.tensor_copy(mw65r[0:I, il, :], mw65_s[0:I, il, :])
            nc.vector.tensor_copy(mw65r[I:I + 1, :, :], mw65_s[I:I + 1, :, :])

            wt_s = consts.tile([I, IL], f32)
            sq_i = nc.scalar.activation(wt_s[:], wsq[:], AF.Sqrt)
            nc.sync.dma_start(WT_d[:], wt_s[:])

            # -------- Weff = W_share @ W_out (all 8 branches: N=16 rhs)
            psWF_cm = tc.tile_pool(name="psWF", bufs=1, space="PSUM")
            psWF = psWF_cm.__enter__()
            weff_r = consts.tile([128, NHC, IL * O], f32r)
            for hc in range(NHC):
                pw = psWF.tile([128, IL * O], f32, tag="wf")
                for kc in range(NHC):
                    nc.tensor.matmul(
                        pw[:], WsTTr[:, kc, hc * 128:(hc + 1) * 128],
                        Wor[:, :, kc, :], start=(kc == 0),
                        stop=(kc == NHC - 1))
                nc.vector.tensor_copy(weff_r[:, hc, :], pw[:])
            # per-branch logit-diff columns of Weff
            weff4 = weff_r.rearrange("p n (i o) -> p n i o", o=O)
            weffd = consts.tile([128, NHC, IL], f32r)
            nc.vector.tensor_sub(weffd[:], weff4[:, :, :, 1], weff4[:, :, :, 0])

            # -------- bias folds: dbias = b_share @ Wod + (bo1 - bo0);
            #          beff0 = b_share @ W_out[0] + b_out[0]
            pb8 = psWF.tile([IL, 2], f32, tag="wf")
            for kc in range(NHC):
                nc.tensor.matmul(pb8[:], Wod[:, :, kc], bshr2[:, kc, :],
                                 start=(kc == 0), stop=(kc == NHC - 1))
            dbias = consts.tile([IL, 1], f32)
            nc.vector.tensor_add(dbias[:], pb8[:, 0:1], bod8_s[:])

            pb0 = psWF.tile([O, 2], f32, tag="wf")
            for kc in range(NHC):
                nc.tensor.matmul(pb0[:], Wor[:, 0, kc, :], bshr2[:, kc, :],
                                 start=(kc == 0), stop=(kc == NHC - 1))
            beff0 = consts.tile([O, 1], f32)
            nc.vector.tensor_add(beff0[:], pb0[:, 0:1], bo01_s[:, 0:1])
            psWF_cm.__exit__(None, None, None)
            psA_cm = tc.tile_pool(name="psA", bufs=3, space="PSUM")
            psA = psA_cm.__enter__()

            # -------- main batched pipeline (gelu table on ACT throughout)
            dstack = consts.tile([IL, B], f32)
            last_gelu = None
            for rep in range(repeats):
                for bt in range(NBT):
                    bsl = slice(bt * BT, (bt + 1) * BT)
                    for il in range(IL):
                        # stage A: h0T chunks, two PSUM banks per ACT op
                        h0_sb = h0_pool.tile([128, NHC, BT], f32r, tag="h0")
                        for hp in range(NHC // 2):
                            pa = psA.tile([128, 2, BT], f32, tag="psA")
                            for h2 in range(2):
                                hc = 2 * hp + h2
                                nc.tensor.matmul(
                                    pa[:, h2, :],
                                    mw65r[:, il, hc * 128:(hc + 1) * 128],
                                    x65r[:, bsl], start=True, stop=True)
                            g_i = nc.scalar.activation(
                                h0_sb[:, 2 * hp:2 * hp + 2, :], pa[:],
                                AF.Gelu)
                            if last_gelu is None:
                                _add_dep_helper(g_i.ins, sq_i.ins, sync=True,
                                                reason="sqrt before gelus")
                            last_gelu = g_i
                        # stage C: logit diff directly from h0
                        pd = psC.tile([1, BT], f32, tag="psC")
                        for kc in range(NHC):
                            nc.tensor.matmul(
                                pd[:], weffd[:, kc, il:il + 1],
                                h0_sb[:, kc, :],
                                start=(kc == 0), stop=(kc == NHC - 1))
                        dal = dall_pool.tile([1, BT], f32, tag="dall")
                        nc.vector.tensor_copy(dal[:], pd[:])
                        # scatter to the branch's partition for batched sigmoid
                        nc.sync.dma_start(dstack[il:il + 1, bsl], dal[:])
                        if il == 0:
                            # branch-0 raw logits (out_layer[:, 0, :])
                            pl = psL.tile([O, BT], f32, tag="psL")
                            for kc in range(NHC):
                                nc.tensor.matmul(
                                    pl[:], weff4[:, kc, 0, :],
                                    h0_sb[:, kc, :],
                                    start=(kc == 0), stop=(kc == NHC - 1))
                            ol = small.tile([O, BT], f32, tag="ol")
                            nc.vector.tensor_scalar_add(
                                ol[:], pl[:], beff0[:, 0:1])
                            nc.sync.dma_start(OL0_d[:, bsl], ol[:])

            # -------- end phase: batched sigmoids (one table swap), sqrt
            def after_gelus(inst):
                _add_dep_helper(inst.ins, last_gelu.ins, sync=True,
                                reason="batch sigmoids after gelus")

            os_t = consts.tile([IL, B], f32)
            s_i = nc.scalar.activation(os_t[:], dstack[:], AF.Sigmoid,
                                       bias=dbias[:])
            after_gelus(s_i)
            nc.sync.dma_start(OutT_d[:], os_t[:])
            nc.sync.dma_start(OSM0_d[1:2, :], os_t[0:1, :])
            om0 = consts.tile([1, B], f32)
            nc.vector.tensor_scalar(
                out=om0[:], in0=os_t[0:1, :], scalar1=-1.0, scalar2=1.0,
                op0=mybir.AluOpType.mult, op1=mybir.AluOpType.add)
            nc.sync.dma_start(OSM0_d[0:1, :], om0[:])

            psA_cm.__exit__(None, None, None)

            if timing:
                tok_s = consts.tile([1, IL], f32)
                nc.sync.dma_start(tok_s[:], tok_d[:])
                nc.sync.dma_start(tok_o[:], tok_s[:])

    nc.compile()
    return nc


# ---------------------------------------------------------------- run

def _make_runner(nc):
    """jit-once runner: takes list of per-core input dicts, returns list of
    per-core output dicts. Modeled on bass2jax.run_bass_via_pjrt."""
    import jax
    from jax.sharding import Mesh, PartitionSpec
    from jax.experimental.shard_map import shard_map
    import concourse.mybir as mybir
    from concourse.bass2jax import (_bass_exec_p, install_neuronx_cc_hook,
                                    partition_id_tensor)

    install_neuronx_cc_hook()

    part_name = nc.partition_id_tensor.name if nc.partition_id_tensor else None
    in_names, out_names, out_avals = [], [], []
    for alloc in nc.m.functions[0].allocations:
        if not isinstance(alloc, mybir.MemoryLocationSet):
            continue
        name = alloc.memorylocations[0].name
        if alloc.kind == "ExternalInput":
            if name != part_name:
                in_names.append(name)
        elif alloc.kind == "ExternalOutput":
            out_names.append(name)
            out_avals.append(jax.core.ShapedArray(
                tuple(alloc.tensor_shape), mybir.dt.np(alloc.dtype)))
    n_params = len(in_names)
    all_names = in_names + out_names + ([part_name] if part_name else [])

    def _body(*args):
        operands = list(args)
        if part_name is not None:
            operands.append(partition_id_tensor())
        outs = _bass_exec_p.bind(
            *operands, out_avals=tuple(out_avals), in_names=tuple(all_names),
            out_names=tuple(out_names), lowering_input_output_aliases=(),
            sim_require_finite=True, sim_require_nnan=True, nc=nc)
        return tuple(outs)

    devices = jax.devices()[:N_CORES]
    mesh = Mesh(np.asarray(devices), ("core",))
    n_outs = len(out_names)
    sharded = jax.jit(shard_map(
        _body, mesh=mesh,
        in_specs=(PartitionSpec("core"),) * (n_params + n_outs),
        out_specs=(PartitionSpec("core"),) * n_outs, check_rep=False))

    zero_shapes = [tuple(a.shape) for a in out_avals]

    def run(in_maps):
        concat_in = [np.concatenate([np.asarray(in_maps[c][n])
                                     for c in range(N_CORES)], axis=0)
                     for n in in_names]
        concat_zeros = [np.zeros((N_CORES * s[0],) + s[1:], np.float32)
                        for s in zero_shapes]
        outs = sharded(*concat_in, *concat_zeros)
        return [
            {name: np.asarray(outs[i]).reshape((N_CORES,) + zero_shapes[i])[c]
             for i, name in enumerate(out_names)}
            for c in range(N_CORES)
        ]

    run.in_names = in_names
    run.out_names = out_names
    run.sharded = sharded
    run.n_params = n_params
    run.zero_shapes = zero_shapes
    return run


def _get_runner(repeats=1):
    key = ("runner", repeats)
    if key not in _CACHE:
        _CACHE[key] = _make_runner(_build(repeats))
    return _CACHE[key]


def _get_timing_runner(repeats, in_map0):
    key = ("timing", repeats)
    if key not in _CACHE:
        _CACHE[key] = _make_runner(_build(repeats, const_ins=in_map0))
    return _CACHE[key]


# ---------------------------------------------------------------- host glue

def _prep_inputs(x, mask_w, mask_b, W_share, b_share, W_out, b_out):
    f = np.float32
    x = np.asarray(x, f)
    mask_w = np.asarray(mask_w, f)
    mask_b = np.asarray(mask_b, f)
    W_share = np.asarray(W_share, f)
    b_share = np.asarray(b_share, f)
    W_out = np.asarray(W_out, f)
    b_out = np.asarray(b_out, f)

    x65 = np.concatenate([x.T, np.ones((1, B), f)], axis=0)          # [65, B]
    WsTT = np.ascontiguousarray(
        W_share.T.reshape(NHC, 128, H).transpose(1, 0, 2))           # [128, 4, 512]
    bshT = np.ascontiguousarray(b_share.reshape(NHC, 128).T)         # [128, 4]

    in_maps = []
    for c in range(N_CORES):
        sl = slice(c * IL, (c + 1) * IL)
        mwT = np.ascontiguousarray(mask_w[sl].transpose(1, 0, 2))    # [64, 8, 512]
        mb1 = np.ascontiguousarray(mask_b[sl].reshape(1, IL, H))     # [1, 8, 512]
        dsc = np.ones((I, IL), f)
        for il in range(IL):
            dsc[c * IL + il, il] = EPS
        WoT = np.ascontiguousarray(
            W_out[sl].reshape(IL, NHC, 128, O).transpose(2, 0, 1, 3))  # [128,8,4,2]
        bod8 = (b_out[sl, 1] - b_out[sl, 0]).reshape(IL, 1).astype(f)
        in_maps.append({
            "x65": x65, "mwT": mwT, "mb1": mb1, "dscale": dsc,
            "WsTT": WsTT, "bshT": bshT, "WoT": WoT,
            "bod8": bod8,
            "bo01": np.ascontiguousarray(b_out[sl].T),               # [2, 8]
        })
    return in_maps


def _assemble(results):
    masked = np.empty((I, I, H), np.float32)
    for c in range(N_CORES):
        masked[c * IL:(c + 1) * IL] = results[c]["masked_o"].transpose(1, 0, 2)
    W = np.concatenate([results[c]["WT"] for c in range(N_CORES)], axis=1)
    Out = np.concatenate([results[c]["OutT"] for c in range(N_CORES)],
                         axis=0).T.copy()
    ol0 = np.ascontiguousarray(results[0]["OL0"].T)
    osm0 = np.ascontiguousarray(results[0]["OSM0"].T)
    return W, Out, ol0, masked, osm0


def kernel(x, mask_w, mask_b, W_share, b_share, W_out, b_out):
    in_maps = _prep_inputs(x, mask_w, mask_b, W_share, b_share, W_out, b_out)
    run = _get_runner(repeats=1)
    results = run(in_maps)
    return _assemble(results)
